# revision 3
# baseline (speedup 1.0000x reference)
"""Trainium2 Bass kernel for nn_Net_3582002725506.

Binarized 4-layer MLP (eval mode):
  fc1(784->3072, sign weights) -> BN -> hardtanh
  fc2(3072->1536, sign both)   -> BN -> hardtanh
  fc3(1536->768, sign both)    -> BN -> hardtanh
  fc4(768->10, float)          -> log_softmax

Strategy: data-parallel batch shard across 8 cores (2048 rows each).
Activations kept transposed on-chip: [features(partitions), batch(free)].

Host-side prep (free, not on HW clock):
  - fc1: x split into 2 fp16 terms, hi = fp16(x) and lo = fp16(x - hi);
    both passes reuse the SAME +-1 fp8 sign-weight tiles (the PE handles
    fp16 subnormals exactly -- probed).  Combined representation error
    ~2^-23|x|, below fp32 PSUM accumulation noise, so numerically
    equivalent to the exact fp32 reference (1 borderline sign flip over
    the whole batch, final rel err ~1.4e-3 vs the 2e-2 budget).
    fc1 runs chunk-major over groups of 4 m-tiles so matmuls unlock as
    weight chunks stream in at startup; the 784 = 6*128 + 16 contraction
    remainder (hi rows 0..15, lo rows 16..31) is replicated across the
    4 partition quadrants and the 4 m-tiles' K=32 tail matmuls run
    CONCURRENTLY on disjoint PE row-groups via tile_position.
  - binarization via DVE is_ge -> u in {0,1} (instead of ScalarE Sign):
    next layer's weights are 2*sign(w) (+-2 exact in fp8) and the
    constant sum(w) row folds into the next threshold / BN3 bias.
    ScalarE then only ever runs Exp/Ln, so its two activation tables
    load once for the whole kernel (no per-tile table thrash).
  - BN1/BN2 + bias folded into per-feature threshold:
    u = (h >= -d), d = b - m + be/a, with sign(a) folded into the next
    layer's sign weights; BN3 kept affine (a3, c3) since fc4 consumes
    real values
  - fc4 bias b4 added on DVE via a broadcast tile; w4 split hi/lo bf16;
    log_softmax per 128-row sub-tile as a short pipelined chain, output
    written per-tile with a single DMA in [t][p][s][10] layout and
    rearranged on host.
"""

import numpy as np
import ml_dtypes

EPS = 1e-5
NCORES = 8
B = 16384
BC = B // NCORES            # 2048 rows per core
NT = 512                    # batch tile (matmul free dim / PSUM bank)
D0, D1, D2, D3 = 784, 3072, 1536, 768
KF = 6                      # full 128-row contraction chunks for fc1
KT = D0 - KF * 128          # 16-row tail
C1, C2, C3 = D1 // 128, D2 // 128, D3 // 128   # 24, 12, 6
MG = 4                      # fc1 m-tile group size (= open PSUM banks)

BF16 = ml_dtypes.bfloat16
FP8 = ml_dtypes.float8_e4m3
FP16 = np.float16


def _chunk3(a2d):
    """[K*128, M] -> [128, K, M] partition-major chunk layout (dtype kept)."""
    k = a2d.shape[0] // 128
    m = a2d.shape[1]
    return np.ascontiguousarray(a2d.reshape(k, 128, m).transpose(1, 0, 2))


def _split2(a):
    hi = a.astype(BF16)
    lo = (a - hi.astype(np.float32)).astype(BF16)
    return hi, lo


def _prep_shared(inp):
    """Host-side preprocessing of weights/BN params (shared by all cores)."""
    out = {}
    a1 = inp["g1"] / np.sqrt(inp["v1"] + EPS)
    a2 = inp["g2"] / np.sqrt(inp["v2"] + EPS)
    a3 = inp["g3"] / np.sqrt(inp["v3"] + EPS)

    # fc1 weights: sign + transpose; 6 full chunks shared by hi/lo passes.
    # 16-row tail: hi rows 0..15 + lo rows 16..31, replicated over the 4
    # partition quadrants for the tile_position-packed tail matmuls.
    s1w_t = np.sign(inp["w1"]).T.astype(np.float32)          # [784, 3072]
    out["w1t"] = _chunk3(s1w_t[:KF * 128].astype(FP8))       # [128, 6, 3072]
    tailblk = np.concatenate([s1w_t[KF * 128:], s1w_t[KF * 128:]], axis=0)
    out["w1tail"] = np.ascontiguousarray(
        np.tile(tailblk, (MG, 1)).astype(FP8))               # [128, 3072]

    # fc2/fc3 weights: 2*sign(w) (exact in fp8) with sign(a_prev) folded;
    # the {0,1} activation trick adds a constant row-sum per feature that
    # folds into the next threshold (d2) / BN3 bias (c3).
    s2w_t = (np.sign(inp["w2"]) * np.sign(a1)[None, :]).T    # [3072, 1536]
    out["w2t"] = _chunk3((2.0 * s2w_t).astype(FP8))          # [128, 24, 1536]
    row2 = s2w_t.sum(axis=0).astype(np.float32)              # [1536]
    s3w_t = (np.sign(inp["w3"]) * np.sign(a2)[None, :]).T    # [1536, 768]
    out["w3t"] = _chunk3((2.0 * s3w_t).astype(FP8))          # [128, 12, 768]
    row3 = s3w_t.sum(axis=0).astype(np.float32)              # [768]

    # fc4: [768, 10] hi/lo -> [128, 6, 20]
    w4hi, w4lo = _split2(inp["w4"].T.astype(np.float32))
    out["w4t"] = _chunk3(np.concatenate([w4hi, w4lo], axis=1))
    # bias as a 128-row broadcast tile for the DVE add
    out["b4bc"] = np.ascontiguousarray(
        np.broadcast_to(inp["b4"].astype(np.float32)[None, :], (128, 10)))

    # thresholds for the is_ge binarization: u = (psum >= thr)
    # fc1: h1 + d1 >= 0  ->  thr1 = -d1
    # fc2: psum2 = h2 + row2  ->  thr2 = row2 - d2
    d1 = (inp["b1"] - inp["m1"] + inp["be1"] / a1).astype(np.float32)
    d2 = (inp["b2"] - inp["m2"] + inp["be2"] / a2).astype(np.float32)
    out["d1"] = np.ascontiguousarray((-d1).reshape(C1, 128).T)  # [128, 24]
    out["d2"] = np.ascontiguousarray((row2 - d2).reshape(C2, 128).T)

    # BN3 affine on psum3 = h3 + row3: a3*(ps - row3) + c3
    c3 = (a3 * (inp["b3"] - inp["m3"] - row3) + inp["be3"]).astype(np.float32)
    out["a3"] = np.ascontiguousarray(a3.astype(np.float32).reshape(C3, 128).T)
    out["c3"] = np.ascontiguousarray(c3.reshape(C3, 128).T)  # [128, 6]
    return out


def _prep_x(x, core):
    """Per-core x shard -> transposed fp16 hi/lo split + packed tail."""
    xs = x[core * BC:(core + 1) * BC]                        # [2048, 784]
    xt = xs.T.astype(np.float32)                             # [784, 2048]
    hi = xt.astype(FP16)
    lo = (xt - hi.astype(np.float32)).astype(FP16)
    d = {}
    d["xh"] = _chunk3(hi[:KF * 128])                         # [128, 6, 2048]
    d["xl"] = _chunk3(lo[:KF * 128])
    tailblk = np.concatenate([hi[KF * 128:], lo[KF * 128:]], axis=0)
    d["xtail"] = np.ascontiguousarray(
        np.tile(tailblk, (MG, 1)))                           # [128, 2048]
    return d


def _build(bc=BC, do_compile=True):
    """Emit the Bass/Tile program (same program for all 8 cores)."""
    import concourse.mybir as mybir
    import concourse.tile as tile
    from concourse import bacc

    dt = mybir.dt
    AF = mybir.ActivationFunctionType
    ALU = mybir.AluOpType
    DR = mybir.MatmulPerfMode.DoubleRow

    nbt = bc // NT
    nsub = NT // 128

    nc = bacc.Bacc(trn_type="TRN2")
    xh_d = nc.declare_dram_parameter("xh", [128, KF, bc], dt.float16, False)
    xl_d = nc.declare_dram_parameter("xl", [128, KF, bc], dt.float16, False)
    xt_d = nc.declare_dram_parameter("xtail", [128, bc], dt.float16, False)
    w1_d = nc.declare_dram_parameter("w1t", [128, KF, D1], dt.float8e4, False)
    w1t_d = nc.declare_dram_parameter("w1tail", [128, D1], dt.float8e4, False)
    w2_d = nc.declare_dram_parameter("w2t", [128, C1, D2], dt.float8e4, False)
    w3_d = nc.declare_dram_parameter("w3t", [128, C2, D3], dt.float8e4, False)
    w4_d = nc.declare_dram_parameter("w4t", [128, C3, 20], dt.bfloat16, False)
    b4_d = nc.declare_dram_parameter("b4bc", [128, 10], dt.float32, False)
    d1_d = nc.declare_dram_parameter("d1", [128, C1], dt.float32, False)
    d2_d = nc.declare_dram_parameter("d2", [128, C2], dt.float32, False)
    a3_d = nc.declare_dram_parameter("a3", [128, C3], dt.float32, False)
    c3_d = nc.declare_dram_parameter("c3", [128, C3], dt.float32, False)
    out_d = nc.declare_dram_parameter("out", [nbt, 128, nsub, 10], dt.float32,
                                      True)

    with tile.TileContext(nc) as tc:
        with (
            tc.tile_pool(name="wpool", bufs=1) as wpool,
            tc.tile_pool(name="vpool", bufs=1) as vpool,
            tc.tile_pool(name="xpool", bufs=2) as xpool,
            tc.tile_pool(name="apool", bufs=1) as apool,
            tc.tile_pool(name="spool", bufs=3) as spool,
            tc.tile_pool(name="pmain", bufs=MG, space="PSUM") as pmain,
            tc.tile_pool(name="plog", bufs=2, space="PSUM") as plog,
            tc.tile_pool(name="pwarm", bufs=2, space="PSUM") as pwarm,
        ):
            # PE warm-up: dummy matmuls (alternating 2 PSUM banks so they
            # pipeline) keep the PE busy while the first DMAs land, opening
            # the HAM clock-gate (1.2 -> 2.4 GHz) before real work starts.
            warm_src = vpool.tile([128, 256], dt.bfloat16)
            nc.vector.memset(warm_src, 0.0)
            for i in range(10):
                wps = pwarm.tile([128, 256], dt.float32, tag="wps",
                                 name=f"wps_{i}")
                nc.tensor.matmul(wps, lhsT=warm_src[:, 0:128], rhs=warm_src,
                                 start=True, stop=True)

            def alloc_x(t):
                tiles = []
                for nm in ("xh", "xl"):
                    tiles.append(xpool.tile([128, KF, NT], dt.float16,
                                            tag=nm, name=f"{nm}_{t}"))
                tiles.append(xpool.tile([128, NT], dt.float16, tag="xt",
                                        name=f"xt_{t}"))
                return tiles

            def dma_x(t, tiles):
                sl = slice(t * NT, (t + 1) * NT)
                for p, src in zip(tiles, (xh_d, xl_d)):
                    nc.sync.dma_start(out=p, in_=src[:, :, sl])
                nc.sync.dma_start(out=tiles[2], in_=xt_d[:, sl])

            def load_x(t):
                tiles = alloc_x(t)
                dma_x(t, tiles)
                return tiles

            # startup-critical-path DMA order: fc1 runs chunk-major, so it
            # needs xh + w1 chunks in order, then xl, tails, thresholds.
            xt = [None] * nbt
            x0 = alloc_x(0)
            xt[0] = x0
            sl0 = slice(0, NT)
            nc.sync.dma_start(out=x0[0], in_=xh_d[:, :, sl0])
            w1s = []
            for c in range(KF):
                w = wpool.tile([128, D1], dt.float8e4, tag=f"w1_{c}",
                               name=f"w1_{c}")
                w1s.append(w)
            nc.sync.dma_start(out=w1s[0], in_=w1_d[:, 0, :])
            nc.sync.dma_start(out=w1s[1], in_=w1_d[:, 1, :])
            nc.sync.dma_start(out=x0[1], in_=xl_d[:, :, sl0])
            nc.sync.dma_start(out=w1s[2], in_=w1_d[:, 2, :])
            nc.sync.dma_start(out=w1s[3], in_=w1_d[:, 3, :])
            nc.sync.dma_start(out=w1s[4], in_=w1_d[:, 4, :])
            nc.sync.dma_start(out=w1s[5], in_=w1_d[:, 5, :])
            nc.sync.dma_start(out=x0[2], in_=xt_d[:, sl0])
            w1tl = wpool.tile([128, D1], dt.float8e4)
            nc.sync.dma_start(out=w1tl, in_=w1t_d[:, :])
            d1s = vpool.tile([128, C1], dt.float32)
            nc.sync.dma_start(out=d1s, in_=d1_d[:, :])
            d2s = vpool.tile([128, C2], dt.float32)
            nc.sync.dma_start(out=d2s, in_=d2_d[:, :])
            a3s = vpool.tile([128, C3], dt.float32)
            nc.sync.dma_start(out=a3s, in_=a3_d[:, :])
            c3s = vpool.tile([128, C3], dt.float32)
            nc.sync.dma_start(out=c3s, in_=c3_d[:, :])
            b4s = vpool.tile([128, 10], dt.float32)
            nc.sync.dma_start(out=b4s, in_=b4_d[:, :])
            w2s = []
            for k in range(C1 // 2):
                w = wpool.tile([128, 2, D2], dt.float8e4, tag=f"w2_{k}",
                               name=f"w2_{k}")
                nc.sync.dma_start(out=w, in_=w2_d[:, 2 * k:2 * k + 2, :])
                w2s.append(w)
            w3s = []
            for k in range(C2 // 2):
                w = wpool.tile([128, 2, D3], dt.float8e4, tag=f"w3_{k}",
                               name=f"w3_{k}")
                nc.sync.dma_start(out=w, in_=w3_d[:, 2 * k:2 * k + 2, :])
                w3s.append(w)
            w4s = wpool.tile([128, C3, 20], dt.bfloat16)
            nc.sync.dma_start(out=w4s, in_=w4_d[:, :, :])

            for t in range(nbt):
                if t + 1 < nbt:
                    xt[t + 1] = load_x(t + 1)
                xh, xl, xtl = xt[t]
                s1 = apool.tile([128, C1, NT], dt.float8e4, tag="s1",
                                name=f"s1_{t}")
                s2 = apool.tile([128, C2, NT], dt.float8e4, tag="s2",
                                name=f"s2_{t}")
                h3 = apool.tile([128, C3, NT], dt.bfloat16, tag="h3",
                                name=f"h3_{t}")

                # fc1: chunk-major over groups of MG m-tiles; hi pass then
                # lo pass (same weights), then the 4 tails concurrently on
                # disjoint PE row-groups, then the 4 binarizations on DVE.
                for g in range(C1 // MG):
                    ms = range(g * MG, (g + 1) * MG)
                    pss = [pmain.tile([128, NT], dt.float32, tag="ps",
                                      name=f"ps1_{t}_{m}") for m in ms]
                    for xpart, first in ((xh, True), (xl, False)):
                        for c in range(KF):
                            for i, m in enumerate(ms):
                                msl = slice(m * 128, (m + 1) * 128)
                                nc.tensor.matmul(pss[i],
                                                 lhsT=w1s[c][:, msl],
                                                 rhs=xpart[:, c, :],
                                                 start=(first and c == 0),
                                                 stop=False)
                    for i, m in enumerate(ms):
                        msl = slice(m * 128, (m + 1) * 128)
                        nc.tensor.matmul(
                            pss[i], lhsT=w1tl[32 * i:32 * i + 32, msl],
                            rhs=xtl[32 * i:32 * i + 32, :],
                            start=False, stop=True,
                            tile_position=(32 * i, 0))
                    for i, m in enumerate(ms):
                        nc.vector.tensor_scalar(out=s1[:, m, :], in0=pss[i],
                                                scalar1=d1s[:, m:m + 1],
                                                scalar2=None, op0=ALU.is_ge)

                # fc2 (fp8 {0,1} x +-2 exact, DoubleRow: 2 K-chunks/matmul)
                for m in range(C2):
                    msl = slice(m * 128, (m + 1) * 128)
                    ps = pmain.tile([128, NT], dt.float32, tag="ps",
                                    name=f"ps2_{t}_{m}")
                    for k in range(C1 // 2):
                        nc.tensor.matmul(ps, lhsT=w2s[k][:, :, msl],
                                         rhs=s1[:, 2 * k:2 * k + 2, :],
                                         start=(k == 0),
                                         stop=(k == C1 // 2 - 1),
                                         perf_mode=DR)
                    nc.vector.tensor_scalar(out=s2[:, m, :], in0=ps,
                                            scalar1=d2s[:, m:m + 1],
                                            scalar2=None, op0=ALU.is_ge)

                # fc3 (DoubleRow) + BN3 affine + hardtanh (bf16 out) on DVE
                for m in range(C3):
                    msl = slice(m * 128, (m + 1) * 128)
                    ps = pmain.tile([128, NT], dt.float32, tag="ps",
                                    name=f"ps3_{t}_{m}")
                    for k in range(C2 // 2):
                        nc.tensor.matmul(ps, lhsT=w3s[k][:, :, msl],
                                         rhs=s2[:, 2 * k:2 * k + 2, :],
                                         start=(k == 0),
                                         stop=(k == C2 // 2 - 1),
                                         perf_mode=DR)
                    bn3 = spool.tile([128, NT], dt.float32, tag="bn3",
                                     name=f"bn3_{t}_{m}")
                    nc.vector.tensor_scalar(out=bn3, in0=ps,
                                            scalar1=a3s[:, m:m + 1],
                                            scalar2=c3s[:, m:m + 1],
                                            op0=ALU.mult, op1=ALU.add)
                    nc.vector.tensor_scalar(out=h3[:, m, :], in0=bn3,
                                            scalar1=-1.0, scalar2=1.0,
                                            op0=ALU.max, op1=ALU.min)

                # fc4 (stationary = activations, moving = w4 hi|lo; b4 on
                # DVE) + log_softmax, as a short pipelined chain per 128-row
                # sub-tile; one output DMA per tile in [p][s][10] layout.
                osb3 = spool.tile([128, nsub, 10], dt.float32, tag="osb",
                                  name=f"osb_{t}", bufs=2)
                for s in range(nsub):
                    ps4 = plog.tile([128, 20], dt.float32, tag="ps4",
                                    name=f"ps4_{t}_{s}")
                    ssl = slice(s * 128, (s + 1) * 128)
                    for c in range(C3):
                        nc.tensor.matmul(ps4, lhsT=h3[:, c, ssl],
                                         rhs=w4s[:, c, :],
                                         start=(c == 0), stop=(c == C3 - 1))
                    # DVE cannot read two PSUM operands; fold the b4 add
                    # into the lo-half staging copy
                    cp1 = spool.tile([128, 10], dt.float32, tag="cp1",
                                     name=f"cp1_{t}_{s}", bufs=2)
                    nc.vector.tensor_tensor(out=cp1, in0=ps4[:, 10:20],
                                            in1=b4s, op=ALU.add)
                    lg = spool.tile([128, 10], dt.float32, tag="lg",
                                    name=f"lg_{t}_{s}", bufs=2)
                    nc.vector.tensor_tensor(out=lg, in0=ps4[:, 0:10],
                                            in1=cp1, op=ALU.add)
                    # logits are bounded (|h3|<=1, small w4), so exp without
                    # max-subtraction is safe; accum_out gives the row sum
                    ex = spool.tile([128, 10], dt.float32, tag="ex",
                                    name=f"ex_{t}_{s}", bufs=2)
                    ssum = spool.tile([128, 1], dt.float32, tag="ssum",
                                      name=f"ssum_{t}_{s}", bufs=2)
                    nc.scalar.activation(out=ex, in_=lg, func=AF.Exp,
                                         accum_out=ssum)
                    lns = spool.tile([128, 1], dt.float32, tag="lns",
                                     name=f"lns_{t}_{s}", bufs=2)
                    nc.scalar.activation(out=lns, in_=ssum, func=AF.Ln)
                    nc.vector.tensor_scalar(out=osb3[:, s, :], in0=lg,
                                            scalar1=lns,
                                            scalar2=None, op0=ALU.subtract)
                nc.sync.dma_start(out=out_d[t, :, :, :], in_=osb3)
    if do_compile:
        # bacc lowering: splits multi-waits into event semaphores (TRN2
        # allows only one sync wait per instruction), register alloc, etc.
        nc.compile()
    return nc


TRACE = False
_LAST_RESULT = [None]


def kernel(**inputs):
    from concourse.bass_utils import run_bass_kernel_spmd

    inp = {k: np.asarray(v) for k, v in inputs.items()}
    x = inp["x"].astype(np.float32)
    shared = _prep_shared(inp)
    nc = _build()
    in_maps = []
    for core in range(NCORES):
        m = _prep_x(x, core)
        m.update(shared)
        in_maps.append(m)
    res = run_bass_kernel_spmd(nc, in_maps, core_ids=list(range(NCORES)),
                               trace=TRACE)
    _LAST_RESULT[0] = res
    parts = []
    for r in res.results:
        o = np.asarray(r["out"], np.float32)       # [nbt, 128, nsub, 10]
        parts.append(o.transpose(0, 2, 1, 3).reshape(BC, 10))
    return np.concatenate(parts, axis=0)


# revision 8
# speedup vs baseline: 1.0513x; 1.0513x over previous
"""Trainium2 Bass kernel for nn_Net_3582002725506.

Binarized 4-layer MLP (eval mode):
  fc1(784->3072, sign weights) -> BN -> hardtanh
  fc2(3072->1536, sign both)   -> BN -> hardtanh
  fc3(1536->768, sign both)    -> BN -> hardtanh
  fc4(768->10, float)          -> log_softmax

Strategy: data-parallel batch shard across 8 cores (2048 rows each).
Activations kept transposed on-chip: [features(partitions), batch(free)].

Host-side prep (free, not on HW clock):
  - fc1: x split into 2 fp16 terms, hi = fp16(x) and lo = fp16(x - hi);
    both passes reuse the SAME +-1 fp8 sign-weight tiles (the PE handles
    fp16 subnormals exactly -- probed).  Combined representation error
    ~2^-23|x|, below fp32 PSUM accumulation noise, so numerically
    equivalent to the exact fp32 reference (1 borderline sign flip over
    the whole batch, final rel err ~1.4e-3 vs the 2e-2 budget).
    fc1 runs chunk-major over groups of 4 m-tiles so matmuls unlock as
    weight chunks stream in at startup; the 784 = 6*128 + 16 contraction
    remainder (hi rows 0..15, lo rows 16..31) is replicated across the
    4 partition quadrants and the 4 m-tiles' K=32 tail matmuls run
    CONCURRENTLY on disjoint PE row-groups via tile_position.
  - binarization via DVE is_ge -> u in {0,1} (instead of ScalarE Sign):
    next layer's weights are 2*sign(w) (+-2 exact in fp8) and the
    constant sum(w) row folds into the next threshold / BN3 bias.
    ScalarE then only ever runs Exp/Ln, so its two activation tables
    load once for the whole kernel (no per-tile table thrash).
  - BN1/BN2 + bias folded into per-feature threshold:
    u = (h >= -d), d = b - m + be/a, with sign(a) folded into the next
    layer's sign weights; BN3 kept affine (a3, c3) since fc4 consumes
    real values
  - fc4 bias b4 added on DVE via a broadcast tile; w4 split hi/lo bf16;
    log_softmax per 128-row sub-tile as a short pipelined chain, output
    written per-tile with a single DMA in [t][p][s][10] layout and
    rearranged on host.
"""

import numpy as np
import ml_dtypes

EPS = 1e-5
NCORES = 8
B = 16384
BC = B // NCORES            # 2048 rows per core
NT = 512                    # batch tile (matmul free dim / PSUM bank)
D0, D1, D2, D3 = 784, 3072, 1536, 768
KF = 6                      # full 128-row contraction chunks for fc1
KT = D0 - KF * 128          # 16-row tail
C1, C2, C3 = D1 // 128, D2 // 128, D3 // 128   # 24, 12, 6
MG = 4                      # fc1 m-tile group size (= open PSUM banks)

BF16 = ml_dtypes.bfloat16
FP8 = ml_dtypes.float8_e4m3
FP16 = np.float16


def _chunk3(a2d):
    """[K*128, M] -> [128, K, M] partition-major chunk layout (dtype kept)."""
    k = a2d.shape[0] // 128
    m = a2d.shape[1]
    return np.ascontiguousarray(a2d.reshape(k, 128, m).transpose(1, 0, 2))


def _split2(a):
    hi = a.astype(BF16)
    lo = (a - hi.astype(np.float32)).astype(BF16)
    return hi, lo


def _prep_shared(inp):
    """Host-side preprocessing of weights/BN params (shared by all cores)."""
    out = {}
    a1 = inp["g1"] / np.sqrt(inp["v1"] + EPS)
    a2 = inp["g2"] / np.sqrt(inp["v2"] + EPS)
    a3 = inp["g3"] / np.sqrt(inp["v3"] + EPS)

    # fc1 weights: sign + transpose; 6 full chunks shared by hi/lo passes.
    # 16-row tail: hi rows 0..15 + lo rows 16..31, replicated over the 4
    # partition quadrants for the tile_position-packed tail matmuls.
    s1w_t = np.sign(inp["w1"]).T.astype(np.float32)          # [784, 3072]
    out["w1t"] = _chunk3(s1w_t[:KF * 128].astype(FP8))       # [128, 6, 3072]
    tailblk = np.concatenate([s1w_t[KF * 128:], s1w_t[KF * 128:]], axis=0)
    out["w1tail"] = np.ascontiguousarray(
        np.tile(tailblk, (MG, 1)).astype(FP8))               # [128, 3072]

    # fc2/fc3 weights: 2*sign(w) (exact in fp8) with sign(a_prev) folded;
    # the {0,1} activation trick adds a constant row-sum per feature that
    # folds into the next threshold (d2) / BN3 bias (c3).
    s2w_t = (np.sign(inp["w2"]) * np.sign(a1)[None, :]).T    # [3072, 1536]
    out["w2t"] = _chunk3((2.0 * s2w_t).astype(FP8))          # [128, 24, 1536]
    row2 = s2w_t.sum(axis=0).astype(np.float32)              # [1536]
    s3w_t = (np.sign(inp["w3"]) * np.sign(a2)[None, :]).T    # [1536, 768]
    out["w3t"] = _chunk3((2.0 * s3w_t).astype(FP8))          # [128, 12, 768]
    row3 = s3w_t.sum(axis=0).astype(np.float32)              # [768]

    # fc4: [768, 10] hi/lo -> [128, 6, 20]
    w4hi, w4lo = _split2(inp["w4"].T.astype(np.float32))
    out["w4t"] = _chunk3(np.concatenate([w4hi, w4lo], axis=1))
    # bias as a 128-row broadcast tile for the DVE add
    out["b4bc"] = np.ascontiguousarray(
        np.broadcast_to(inp["b4"].astype(np.float32)[None, :], (128, 10)))

    # thresholds for the is_ge binarization: u = (psum >= thr)
    # fc1: h1 + d1 >= 0  ->  thr1 = -d1
    # fc2: psum2 = h2 + row2  ->  thr2 = row2 - d2
    d1 = (inp["b1"] - inp["m1"] + inp["be1"] / a1).astype(np.float32)
    d2 = (inp["b2"] - inp["m2"] + inp["be2"] / a2).astype(np.float32)
    out["d1"] = np.ascontiguousarray((-d1).reshape(C1, 128).T)  # [128, 24]
    out["d2"] = np.ascontiguousarray((row2 - d2).reshape(C2, 128).T)

    # BN3 affine on psum3 = h3 + row3: a3*(ps - row3) + c3
    c3 = (a3 * (inp["b3"] - inp["m3"] - row3) + inp["be3"]).astype(np.float32)
    out["a3"] = np.ascontiguousarray(a3.astype(np.float32).reshape(C3, 128).T)
    out["c3"] = np.ascontiguousarray(c3.reshape(C3, 128).T)  # [128, 6]
    return out


def _prep_x(x, core):
    """Per-core x shard -> transposed fp16 hi/lo split + packed tail."""
    xs = x[core * BC:(core + 1) * BC]                        # [2048, 784]
    xt = xs.T.astype(np.float32)                             # [784, 2048]
    hi = xt.astype(FP16)
    lo = (xt - hi.astype(np.float32)).astype(FP16)
    d = {}
    d["xh"] = _chunk3(hi[:KF * 128])                         # [128, 6, 2048]
    d["xl"] = _chunk3(lo[:KF * 128])
    tailblk = np.concatenate([hi[KF * 128:], lo[KF * 128:]], axis=0)
    d["xtail"] = np.ascontiguousarray(
        np.tile(tailblk, (MG, 1)))                           # [128, 2048]
    return d


def _build(bc=BC, do_compile=True):
    """Emit the Bass/Tile program (same program for all 8 cores)."""
    import concourse.mybir as mybir
    import concourse.tile as tile
    from concourse import bacc

    dt = mybir.dt
    AF = mybir.ActivationFunctionType
    ALU = mybir.AluOpType
    DR = mybir.MatmulPerfMode.DoubleRow

    nbt = bc // NT
    nsub = NT // 128

    nc = bacc.Bacc(trn_type="TRN2")
    xh_d = nc.declare_dram_parameter("xh", [128, KF, bc], dt.float16, False)
    xl_d = nc.declare_dram_parameter("xl", [128, KF, bc], dt.float16, False)
    xt_d = nc.declare_dram_parameter("xtail", [128, bc], dt.float16, False)
    w1_d = nc.declare_dram_parameter("w1t", [128, KF, D1], dt.float8e4, False)
    w1t_d = nc.declare_dram_parameter("w1tail", [128, D1], dt.float8e4, False)
    w2_d = nc.declare_dram_parameter("w2t", [128, C1, D2], dt.float8e4, False)
    w3_d = nc.declare_dram_parameter("w3t", [128, C2, D3], dt.float8e4, False)
    w4_d = nc.declare_dram_parameter("w4t", [128, C3, 20], dt.bfloat16, False)
    b4_d = nc.declare_dram_parameter("b4bc", [128, 10], dt.float32, False)
    d1_d = nc.declare_dram_parameter("d1", [128, C1], dt.float32, False)
    d2_d = nc.declare_dram_parameter("d2", [128, C2], dt.float32, False)
    a3_d = nc.declare_dram_parameter("a3", [128, C3], dt.float32, False)
    c3_d = nc.declare_dram_parameter("c3", [128, C3], dt.float32, False)
    out_d = nc.declare_dram_parameter("out", [nbt, 128, nsub, 10], dt.float32,
                                      True)

    with tile.TileContext(nc) as tc:
        with (
            tc.tile_pool(name="wpool", bufs=1) as wpool,
            tc.tile_pool(name="vpool", bufs=1) as vpool,
            tc.tile_pool(name="xpool", bufs=2) as xpool,
            tc.tile_pool(name="apool", bufs=1) as apool,
            tc.tile_pool(name="spool", bufs=3) as spool,
            tc.tile_pool(name="pmain", bufs=6, space="PSUM") as pmain,
            tc.tile_pool(name="plog", bufs=2, space="PSUM") as plog,
        ):
            # PE warm-up: dummy matmuls (rotating the 6 main PSUM banks so
            # they pipeline) keep the PE busy while the first DMAs land,
            # opening the HAM clock-gate (1.2 -> 2.4 GHz) before real work.
            warm_src = vpool.tile([128, NT], dt.bfloat16)
            nc.vector.memset(warm_src, 0.0)
            for i in range(8):
                wps = pmain.tile([128, NT], dt.float32, tag="ps",
                                 name=f"wps_{i}")
                nc.tensor.matmul(wps, lhsT=warm_src[:, 0:128], rhs=warm_src,
                                 start=True, stop=True)

            def alloc_x(t):
                tiles = []
                for nm in ("xh", "xl"):
                    tiles.append(xpool.tile([128, KF, NT], dt.float16,
                                            tag=nm, name=f"{nm}_{t}"))
                tiles.append(xpool.tile([128, NT], dt.float16, tag="xt",
                                        name=f"xt_{t}"))
                return tiles

            def dma_x(t, tiles):
                sl = slice(t * NT, (t + 1) * NT)
                for p, src in zip(tiles, (xh_d, xl_d)):
                    nc.sync.dma_start(out=p, in_=src[:, :, sl])
                nc.sync.dma_start(out=tiles[2], in_=xt_d[:, sl])

            def load_x(t):
                tiles = alloc_x(t)
                dma_x(t, tiles)
                return tiles

            # startup-critical-path DMA order: fc1 runs chunk-major, so it
            # needs xh + w1 chunks in order, then xl, tails, thresholds.
            xt = [None] * nbt
            x0 = alloc_x(0)
            xt[0] = x0
            sl0 = slice(0, NT)
            nc.sync.dma_start(out=x0[0], in_=xh_d[:, :, sl0])
            nc.sync.dma_start(out=x0[2], in_=xt_d[:, sl0])
            w1tl = wpool.tile([128, D1], dt.float8e4)
            nc.sync.dma_start(out=w1tl, in_=w1t_d[:, :])
            w1s = []
            for c in range(KF):
                w = wpool.tile([128, D1], dt.float8e4, tag=f"w1_{c}",
                               name=f"w1_{c}")
                w1s.append(w)
            nc.sync.dma_start(out=w1s[0], in_=w1_d[:, 0, :])
            nc.sync.dma_start(out=w1s[1], in_=w1_d[:, 1, :])
            nc.sync.dma_start(out=x0[1], in_=xl_d[:, :, sl0])
            nc.sync.dma_start(out=w1s[2], in_=w1_d[:, 2, :])
            nc.sync.dma_start(out=w1s[3], in_=w1_d[:, 3, :])
            nc.sync.dma_start(out=w1s[4], in_=w1_d[:, 4, :])
            nc.sync.dma_start(out=w1s[5], in_=w1_d[:, 5, :])
            d1s = vpool.tile([128, C1], dt.float32)
            nc.sync.dma_start(out=d1s, in_=d1_d[:, :])
            d2s = vpool.tile([128, C2], dt.float32)
            nc.sync.dma_start(out=d2s, in_=d2_d[:, :])
            a3s = vpool.tile([128, C3], dt.float32)
            nc.sync.dma_start(out=a3s, in_=a3_d[:, :])
            c3s = vpool.tile([128, C3], dt.float32)
            nc.sync.dma_start(out=c3s, in_=c3_d[:, :])
            b4s = vpool.tile([128, 10], dt.float32)
            nc.sync.dma_start(out=b4s, in_=b4_d[:, :])
            w2s = []
            for k in range(C1 // 2):
                w = wpool.tile([128, 2, D2], dt.float8e4, tag=f"w2_{k}",
                               name=f"w2_{k}")
                nc.sync.dma_start(out=w, in_=w2_d[:, 2 * k:2 * k + 2, :])
                w2s.append(w)
            w3s = []
            for k in range(C2 // 2):
                w = wpool.tile([128, 2, D3], dt.float8e4, tag=f"w3_{k}",
                               name=f"w3_{k}")
                nc.sync.dma_start(out=w, in_=w3_d[:, 2 * k:2 * k + 2, :])
                w3s.append(w)
            w4s = wpool.tile([128, C3, 20], dt.bfloat16)
            nc.sync.dma_start(out=w4s, in_=w4_d[:, :, :])

            for t in range(nbt):
                if t + 1 < nbt:
                    xt[t + 1] = load_x(t + 1)
                xh, xl, xtl = xt[t]
                s1 = apool.tile([128, C1, NT], dt.float8e4, tag="s1",
                                name=f"s1_{t}")
                s2 = apool.tile([128, C2, NT], dt.float8e4, tag="s2",
                                name=f"s2_{t}")
                h3 = apool.tile([128, C3, NT], dt.bfloat16, tag="h3",
                                name=f"h3_{t}")

                # fc1 over groups of MG m-tiles: the 4 K=32 tail matmuls go
                # FIRST (start=True), packed concurrently on disjoint PE
                # row-groups via tile_position; then the hi/lo main matmuls.
                # Steady state runs m-major so each m-tile's PSUM closes
                # early and its DVE binarization overlaps the next m-tile's
                # matmuls (no bank-recycle stall); the startup group runs
                # chunk-major so matmuls unlock as weight chunks stream in.
                for g in range(C1 // MG):
                    ms = range(g * MG, (g + 1) * MG)
                    pss = [pmain.tile([128, NT], dt.float32, tag="ps",
                                      name=f"ps1_{t}_{m}") for m in ms]
                    for i, m in enumerate(ms):
                        msl = slice(m * 128, (m + 1) * 128)
                        nc.tensor.matmul(
                            pss[i], lhsT=w1tl[32 * i:32 * i + 32, msl],
                            rhs=xtl[32 * i:32 * i + 32, :],
                            start=True, stop=False,
                            tile_position=(32 * i, 0))
                    if t == 0 and g == 0:
                        for xpart, last in ((xh, False), (xl, True)):
                            for c in range(KF):
                                for i, m in enumerate(ms):
                                    msl = slice(m * 128, (m + 1) * 128)
                                    nc.tensor.matmul(
                                        pss[i], lhsT=w1s[c][:, msl],
                                        rhs=xpart[:, c, :], start=False,
                                        stop=(last and c == KF - 1))
                    else:
                        for i, m in enumerate(ms):
                            msl = slice(m * 128, (m + 1) * 128)
                            for xpart, last in ((xh, False), (xl, True)):
                                for c in range(KF):
                                    nc.tensor.matmul(
                                        pss[i], lhsT=w1s[c][:, msl],
                                        rhs=xpart[:, c, :], start=False,
                                        stop=(last and c == KF - 1))
                            nc.vector.tensor_scalar(
                                out=s1[:, m, :], in0=pss[i],
                                scalar1=d1s[:, m:m + 1],
                                scalar2=None, op0=ALU.is_ge)
                    if t == 0 and g == 0:
                        for i, m in enumerate(ms):
                            nc.vector.tensor_scalar(
                                out=s1[:, m, :], in0=pss[i],
                                scalar1=d1s[:, m:m + 1],
                                scalar2=None, op0=ALU.is_ge)

                # fc2 (fp8 {0,1} x +-2 exact, DoubleRow: 2 K-chunks/matmul)
                for m in range(C2):
                    msl = slice(m * 128, (m + 1) * 128)
                    ps = pmain.tile([128, NT], dt.float32, tag="ps",
                                    name=f"ps2_{t}_{m}")
                    for k in range(C1 // 2):
                        nc.tensor.matmul(ps, lhsT=w2s[k][:, :, msl],
                                         rhs=s1[:, 2 * k:2 * k + 2, :],
                                         start=(k == 0),
                                         stop=(k == C1 // 2 - 1),
                                         perf_mode=DR)
                    nc.vector.tensor_scalar(out=s2[:, m, :], in0=ps,
                                            scalar1=d2s[:, m:m + 1],
                                            scalar2=None, op0=ALU.is_ge)

                # fc3 (DoubleRow) + BN3 affine + hardtanh (bf16 out) on DVE
                for m in range(C3):
                    msl = slice(m * 128, (m + 1) * 128)
                    ps = pmain.tile([128, NT], dt.float32, tag="ps",
                                    name=f"ps3_{t}_{m}")
                    for k in range(C2 // 2):
                        nc.tensor.matmul(ps, lhsT=w3s[k][:, :, msl],
                                         rhs=s2[:, 2 * k:2 * k + 2, :],
                                         start=(k == 0),
                                         stop=(k == C2 // 2 - 1),
                                         perf_mode=DR)
                    bn3 = spool.tile([128, NT], dt.float32, tag="bn3",
                                     name=f"bn3_{t}_{m}")
                    nc.vector.tensor_scalar(out=bn3, in0=ps,
                                            scalar1=a3s[:, m:m + 1],
                                            scalar2=c3s[:, m:m + 1],
                                            op0=ALU.mult, op1=ALU.add)
                    nc.vector.tensor_scalar(out=h3[:, m, :], in0=bn3,
                                            scalar1=-1.0, scalar2=1.0,
                                            op0=ALU.max, op1=ALU.min)

                # fc4 (stationary = activations, moving = w4 hi|lo; b4 on
                # DVE) + log_softmax, as a short pipelined chain per 128-row
                # sub-tile; one output DMA per tile in [p][s][10] layout.
                osb3 = spool.tile([128, nsub, 10], dt.float32, tag="osb",
                                  name=f"osb_{t}", bufs=2)
                for s in range(nsub):
                    ps4 = plog.tile([128, 20], dt.float32, tag="ps4",
                                    name=f"ps4_{t}_{s}")
                    ssl = slice(s * 128, (s + 1) * 128)
                    for c in range(C3):
                        nc.tensor.matmul(ps4, lhsT=h3[:, c, ssl],
                                         rhs=w4s[:, c, :],
                                         start=(c == 0), stop=(c == C3 - 1))
                    # DVE cannot read two PSUM operands; fold the b4 add
                    # into the lo-half staging copy
                    cp1 = spool.tile([128, 10], dt.float32, tag="cp1",
                                     name=f"cp1_{t}_{s}", bufs=2)
                    nc.vector.tensor_tensor(out=cp1, in0=ps4[:, 10:20],
                                            in1=b4s, op=ALU.add)
                    lg = spool.tile([128, 10], dt.float32, tag="lg",
                                    name=f"lg_{t}_{s}", bufs=2)
                    nc.vector.tensor_tensor(out=lg, in0=ps4[:, 0:10],
                                            in1=cp1, op=ALU.add)
                    # logits are bounded (|h3|<=1, small w4), so exp without
                    # max-subtraction is safe; accum_out gives the row sum
                    ex = spool.tile([128, 10], dt.float32, tag="ex",
                                    name=f"ex_{t}_{s}", bufs=2)
                    ssum = spool.tile([128, 1], dt.float32, tag="ssum",
                                      name=f"ssum_{t}_{s}", bufs=2)
                    nc.scalar.activation(out=ex, in_=lg, func=AF.Exp,
                                         accum_out=ssum)
                    lns = spool.tile([128, 1], dt.float32, tag="lns",
                                     name=f"lns_{t}_{s}", bufs=2)
                    nc.scalar.activation(out=lns, in_=ssum, func=AF.Ln)
                    nc.vector.tensor_scalar(out=osb3[:, s, :], in0=lg,
                                            scalar1=lns,
                                            scalar2=None, op0=ALU.subtract)
                nc.sync.dma_start(out=out_d[t, :, :, :], in_=osb3)
    if do_compile:
        # bacc lowering: splits multi-waits into event semaphores (TRN2
        # allows only one sync wait per instruction), register alloc, etc.
        nc.compile()
    return nc


TRACE = False
_LAST_RESULT = [None]


def kernel(**inputs):
    from concourse.bass_utils import run_bass_kernel_spmd

    inp = {k: np.asarray(v) for k, v in inputs.items()}
    x = inp["x"].astype(np.float32)
    shared = _prep_shared(inp)
    nc = _build()
    in_maps = []
    for core in range(NCORES):
        m = _prep_x(x, core)
        m.update(shared)
        in_maps.append(m)
    res = run_bass_kernel_spmd(nc, in_maps, core_ids=list(range(NCORES)),
                               trace=TRACE)
    _LAST_RESULT[0] = res
    parts = []
    for r in res.results:
        o = np.asarray(r["out"], np.float32)       # [nbt, 128, nsub, 10]
        parts.append(o.transpose(0, 2, 1, 3).reshape(BC, 10))
    return np.concatenate(parts, axis=0)


# revision 13
# speedup vs baseline: 1.0672x; 1.0151x over previous
"""Trainium2 Bass kernel for nn_Net_3582002725506.

Binarized 4-layer MLP (eval mode):
  fc1(784->3072, sign weights) -> BN -> hardtanh
  fc2(3072->1536, sign both)   -> BN -> hardtanh
  fc3(1536->768, sign both)    -> BN -> hardtanh
  fc4(768->10, float)          -> log_softmax

Strategy: data-parallel batch shard across 8 cores (2048 rows each).
Activations kept transposed on-chip: [features(partitions), batch(free)].

Host-side prep (free, not on HW clock):
  - fc1: x split into 2 fp16 terms, hi = fp16(x) and lo = fp16(x - hi);
    both passes reuse the SAME +-1 fp8 sign-weight tiles (the PE handles
    fp16 subnormals exactly -- probed).  Combined representation error
    ~2^-23|x|, below fp32 PSUM accumulation noise, so numerically
    equivalent to the exact fp32 reference (1 borderline sign flip over
    the whole batch, final rel err ~1.4e-3 vs the 2e-2 budget).
    fc1 runs chunk-major over groups of 4 m-tiles so matmuls unlock as
    weight chunks stream in at startup; the 784 = 6*128 + 16 contraction
    remainder (hi rows 0..15, lo rows 16..31) is replicated across the
    4 partition quadrants and the 4 m-tiles' K=32 tail matmuls run
    CONCURRENTLY on disjoint PE row-groups via tile_position.
  - binarization via DVE is_ge -> u in {0,1} (instead of ScalarE Sign):
    next layer's weights are 2*sign(w) (+-2 exact in fp8) and the
    constant sum(w) row folds into the next threshold / BN3 bias.
    ScalarE then only ever runs Exp/Ln, so its two activation tables
    load once for the whole kernel (no per-tile table thrash).
  - BN1/BN2 + bias folded into per-feature threshold:
    u = (h >= -d), d = b - m + be/a, with sign(a) folded into the next
    layer's sign weights; BN3 kept affine (a3, c3) since fc4 consumes
    real values
  - fc4 bias b4 added on DVE via a broadcast tile; w4 split hi/lo bf16;
    log_softmax per 128-row sub-tile as a short pipelined chain, output
    written per-tile with a single DMA in [t][p][s][10] layout and
    rearranged on host.
"""

import numpy as np
import ml_dtypes

EPS = 1e-5
NCORES = 8
B = 16384
BC = B // NCORES            # 2048 rows per core
NT = 512                    # batch tile (matmul free dim / PSUM bank)
D0, D1, D2, D3 = 784, 3072, 1536, 768
KF = 6                      # full 128-row contraction chunks for fc1
KT = D0 - KF * 128          # 16-row tail
C1, C2, C3 = D1 // 128, D2 // 128, D3 // 128   # 24, 12, 6
MG = 4                      # fc1 m-tile group size (= open PSUM banks)

BF16 = ml_dtypes.bfloat16
FP8 = ml_dtypes.float8_e4m3
FP16 = np.float16


def _chunk3(a2d):
    """[K*128, M] -> [128, K, M] partition-major chunk layout (dtype kept)."""
    k = a2d.shape[0] // 128
    m = a2d.shape[1]
    return np.ascontiguousarray(a2d.reshape(k, 128, m).transpose(1, 0, 2))


def _split2(a):
    hi = a.astype(BF16)
    lo = (a - hi.astype(np.float32)).astype(BF16)
    return hi, lo


def _prep_shared(inp):
    """Host-side preprocessing of weights/BN params (shared by all cores)."""
    out = {}
    a1 = inp["g1"] / np.sqrt(inp["v1"] + EPS)
    a2 = inp["g2"] / np.sqrt(inp["v2"] + EPS)
    a3 = inp["g3"] / np.sqrt(inp["v3"] + EPS)

    # fc1 weights: sign + transpose; 6 full chunks shared by hi/lo passes.
    # 16-row tail: hi rows 0..15 + lo rows 16..31, replicated over the 4
    # partition quadrants for the tile_position-packed tail matmuls.
    s1w_t = np.sign(inp["w1"]).T.astype(np.float32)          # [784, 3072]
    out["w1t"] = _chunk3(s1w_t[:KF * 128].astype(FP8))       # [128, 6, 3072]
    tailblk = np.concatenate([s1w_t[KF * 128:], s1w_t[KF * 128:]], axis=0)
    out["w1tail"] = np.ascontiguousarray(
        np.tile(tailblk, (MG, 1)).astype(FP8))               # [128, 3072]

    # fc2/fc3 weights: 2*sign(w) (exact in fp8) with sign(a_prev) folded;
    # the {0,1} activation trick adds a constant row-sum per feature that
    # folds into the next threshold (d2) / BN3 bias (c3).
    s2w_t = (np.sign(inp["w2"]) * np.sign(a1)[None, :]).T    # [3072, 1536]
    out["w2t"] = _chunk3((2.0 * s2w_t).astype(FP8))          # [128, 24, 1536]
    row2 = s2w_t.sum(axis=0).astype(np.float32)              # [1536]
    s3w_t = (np.sign(inp["w3"]) * np.sign(a2)[None, :]).T    # [1536, 768]
    out["w3t"] = _chunk3((2.0 * s3w_t).astype(FP8))          # [128, 12, 768]
    row3 = s3w_t.sum(axis=0).astype(np.float32)              # [768]

    # fc4: [768, 10] hi/lo -> [128, 6, 20]
    w4hi, w4lo = _split2(inp["w4"].T.astype(np.float32))
    out["w4t"] = _chunk3(np.concatenate([w4hi, w4lo], axis=1))
    # bias as a 128-row broadcast tile for the DVE add
    out["b4bc"] = np.ascontiguousarray(
        np.broadcast_to(inp["b4"].astype(np.float32)[None, :], (128, 10)))

    # thresholds for the is_ge binarization: u = (psum >= thr)
    # fc1: h1 + d1 >= 0  ->  thr1 = -d1
    # fc2: psum2 = h2 + row2  ->  thr2 = row2 - d2
    d1 = (inp["b1"] - inp["m1"] + inp["be1"] / a1).astype(np.float32)
    d2 = (inp["b2"] - inp["m2"] + inp["be2"] / a2).astype(np.float32)
    out["d1"] = np.ascontiguousarray((-d1).reshape(C1, 128).T)  # [128, 24]
    out["d2"] = np.ascontiguousarray((row2 - d2).reshape(C2, 128).T)

    # BN3 affine on psum3 = h3 + row3: a3*(ps - row3) + c3
    c3 = (a3 * (inp["b3"] - inp["m3"] - row3) + inp["be3"]).astype(np.float32)
    out["a3"] = np.ascontiguousarray(a3.astype(np.float32).reshape(C3, 128).T)
    out["c3"] = np.ascontiguousarray(c3.reshape(C3, 128).T)  # [128, 6]
    return out


def _prep_x(x, core):
    """Per-core x shard -> transposed fp16 hi/lo split + packed tail."""
    xs = x[core * BC:(core + 1) * BC]                        # [2048, 784]
    xt = xs.T.astype(np.float32)                             # [784, 2048]
    hi = xt.astype(FP16)
    lo = (xt - hi.astype(np.float32)).astype(FP16)
    d = {}
    d["xh"] = _chunk3(hi[:KF * 128])                         # [128, 6, 2048]
    d["xl"] = _chunk3(lo[:KF * 128])
    tailblk = np.concatenate([hi[KF * 128:], lo[KF * 128:]], axis=0)
    d["xtail"] = np.ascontiguousarray(
        np.tile(tailblk, (MG, 1)))                           # [128, 2048]
    return d


def _build(bc=BC, do_compile=True):
    """Emit the Bass/Tile program (same program for all 8 cores)."""
    import concourse.mybir as mybir
    import concourse.tile as tile
    from concourse import bacc

    dt = mybir.dt
    AF = mybir.ActivationFunctionType
    ALU = mybir.AluOpType
    DR = mybir.MatmulPerfMode.DoubleRow

    nbt = bc // NT
    nsub = NT // 128

    nc = bacc.Bacc(trn_type="TRN2")
    xh_d = nc.declare_dram_parameter("xh", [128, KF, bc], dt.float16, False)
    xl_d = nc.declare_dram_parameter("xl", [128, KF, bc], dt.float16, False)
    xt_d = nc.declare_dram_parameter("xtail", [128, bc], dt.float16, False)
    w1_d = nc.declare_dram_parameter("w1t", [128, KF, D1], dt.float8e4, False)
    w1t_d = nc.declare_dram_parameter("w1tail", [128, D1], dt.float8e4, False)
    w2_d = nc.declare_dram_parameter("w2t", [128, C1, D2], dt.float8e4, False)
    w3_d = nc.declare_dram_parameter("w3t", [128, C2, D3], dt.float8e4, False)
    w4_d = nc.declare_dram_parameter("w4t", [128, C3, 20], dt.bfloat16, False)
    b4_d = nc.declare_dram_parameter("b4bc", [128, 10], dt.float32, False)
    d1_d = nc.declare_dram_parameter("d1", [128, C1], dt.float32, False)
    d2_d = nc.declare_dram_parameter("d2", [128, C2], dt.float32, False)
    a3_d = nc.declare_dram_parameter("a3", [128, C3], dt.float32, False)
    c3_d = nc.declare_dram_parameter("c3", [128, C3], dt.float32, False)
    out_d = nc.declare_dram_parameter("out", [nbt, 128, nsub, 10], dt.float32,
                                      True)

    with tile.TileContext(nc) as tc:
        with (
            tc.tile_pool(name="wpool", bufs=1) as wpool,
            tc.tile_pool(name="vpool", bufs=1) as vpool,
            tc.tile_pool(name="xpool", bufs=2) as xpool,
            tc.tile_pool(name="apool", bufs=1) as apool,
            tc.tile_pool(name="spool", bufs=3) as spool,
            tc.tile_pool(name="pmain", bufs=5, space="PSUM") as pmain,
            tc.tile_pool(name="plog", bufs=3, space="PSUM") as plog,
        ):
            # PE warm-up: dummy matmuls (rotating the main PSUM banks so
            # they pipeline) keep the PE busy while the first DMAs land,
            # opening the HAM clock-gate (1.2 -> 2.4 GHz) before real work.
            warm_src = vpool.tile([128, NT], dt.bfloat16)
            nc.vector.memset(warm_src, 0.0)
            for i in range(10):
                wps = pmain.tile([128, NT], dt.float32, tag="ps",
                                 name=f"wps_{i}")
                nc.tensor.matmul(wps, lhsT=warm_src[:, 0:128], rhs=warm_src,
                                 start=True, stop=True)

            def alloc_x(t):
                tiles = []
                for nm in ("xh", "xl"):
                    tiles.append(xpool.tile([128, KF, NT], dt.float16,
                                            tag=nm, name=f"{nm}_{t}"))
                tiles.append(xpool.tile([128, NT], dt.float16, tag="xt",
                                        name=f"xt_{t}"))
                return tiles

            def dma_x(t, tiles):
                sl = slice(t * NT, (t + 1) * NT)
                for p, src in zip(tiles, (xh_d, xl_d)):
                    nc.sync.dma_start(out=p, in_=src[:, :, sl])
                nc.sync.dma_start(out=tiles[2], in_=xt_d[:, sl])

            def load_x(t):
                tiles = alloc_x(t)
                dma_x(t, tiles)
                return tiles

            # startup-critical-path DMA order: fc1 runs chunk-major, so it
            # needs xh + w1 chunks in order, then xl, tails, thresholds.
            xt = [None] * nbt
            x0 = alloc_x(0)
            xt[0] = x0
            sl0 = slice(0, NT)
            nc.sync.dma_start(out=x0[0], in_=xh_d[:, :, sl0])
            nc.sync.dma_start(out=x0[2], in_=xt_d[:, sl0])
            w1tl = wpool.tile([128, D1], dt.float8e4)
            nc.sync.dma_start(out=w1tl, in_=w1t_d[:, :])
            w1s = []
            for c in range(KF):
                w = wpool.tile([128, D1], dt.float8e4, tag=f"w1_{c}",
                               name=f"w1_{c}")
                w1s.append(w)
            nc.sync.dma_start(out=w1s[0], in_=w1_d[:, 0, :])
            nc.sync.dma_start(out=w1s[1], in_=w1_d[:, 1, :])
            nc.sync.dma_start(out=x0[1], in_=xl_d[:, :, sl0])
            nc.sync.dma_start(out=w1s[2], in_=w1_d[:, 2, :])
            nc.sync.dma_start(out=w1s[3], in_=w1_d[:, 3, :])
            nc.sync.dma_start(out=w1s[4], in_=w1_d[:, 4, :])
            nc.sync.dma_start(out=w1s[5], in_=w1_d[:, 5, :])
            # startup group-0 matmul order: hi c0, c1, then lo c0, c1 (after
            # xl lands), then hi/lo alternating per chunk -- keeps the PE
            # fed at the pace weight chunks stream in.
            g0_order = [(0, 0), (0, 1), (1, 0), (1, 1)] + [
                (hl, c) for c in range(2, KF) for hl in (0, 1)]
            d1s = vpool.tile([128, C1], dt.float32)
            nc.sync.dma_start(out=d1s, in_=d1_d[:, :])
            d2s = vpool.tile([128, C2], dt.float32)
            nc.sync.dma_start(out=d2s, in_=d2_d[:, :])
            a3s = vpool.tile([128, C3], dt.float32)
            nc.sync.dma_start(out=a3s, in_=a3_d[:, :])
            c3s = vpool.tile([128, C3], dt.float32)
            nc.sync.dma_start(out=c3s, in_=c3_d[:, :])
            b4s = vpool.tile([128, 10], dt.float32)
            nc.sync.dma_start(out=b4s, in_=b4_d[:, :])
            w2s = []
            for k in range(C1 // 2):
                w = wpool.tile([128, 2, D2], dt.float8e4, tag=f"w2_{k}",
                               name=f"w2_{k}")
                nc.sync.dma_start(out=w, in_=w2_d[:, 2 * k:2 * k + 2, :])
                w2s.append(w)
            w3s = []
            for k in range(C2 // 2):
                w = wpool.tile([128, 2, D3], dt.float8e4, tag=f"w3_{k}",
                               name=f"w3_{k}")
                nc.sync.dma_start(out=w, in_=w3_d[:, 2 * k:2 * k + 2, :])
                w3s.append(w)
            w4s = wpool.tile([128, C3, 20], dt.bfloat16)
            nc.sync.dma_start(out=w4s, in_=w4_d[:, :, :])

            for t in range(nbt):
                if t + 1 < nbt:
                    xt[t + 1] = load_x(t + 1)
                xh, xl, xtl = xt[t]
                s1 = apool.tile([128, C1, NT], dt.float8e4, tag="s1",
                                name=f"s1_{t}")
                s2 = apool.tile([128, C2, NT], dt.float8e4, tag="s2",
                                name=f"s2_{t}")
                h3 = apool.tile([128, C3, NT], dt.bfloat16, tag="h3",
                                name=f"h3_{t}")

                # fc1 over groups of MG m-tiles: the 4 K=32 tail matmuls go
                # FIRST (start=True), packed concurrently on disjoint PE
                # row-groups via tile_position; then the hi/lo main matmuls.
                # Steady state runs m-major so each m-tile's PSUM closes
                # early and its DVE binarization overlaps the next m-tile's
                # matmuls (no bank-recycle stall); the startup group runs
                # chunk-major so matmuls unlock as weight chunks stream in.
                for g in range(C1 // MG):
                    ms = range(g * MG, (g + 1) * MG)
                    pss = [pmain.tile([128, NT], dt.float32, tag="ps",
                                      name=f"ps1_{t}_{m}") for m in ms]
                    for i, m in enumerate(ms):
                        msl = slice(m * 128, (m + 1) * 128)
                        nc.tensor.matmul(
                            pss[i], lhsT=w1tl[32 * i:32 * i + 32, msl],
                            rhs=xtl[32 * i:32 * i + 32, :],
                            start=True, stop=False,
                            tile_position=(32 * i, 0))
                    if t == 0 and g == 0:
                        for hl, c in g0_order:
                            xpart = xh if hl == 0 else xl
                            last = (hl, c) == g0_order[-1]
                            for i, m in enumerate(ms):
                                msl = slice(m * 128, (m + 1) * 128)
                                nc.tensor.matmul(
                                    pss[i], lhsT=w1s[c][:, msl],
                                    rhs=xpart[:, c, :], start=False,
                                    stop=last)
                    else:
                        for i, m in enumerate(ms):
                            msl = slice(m * 128, (m + 1) * 128)
                            for xpart, last in ((xh, False), (xl, True)):
                                for c in range(KF):
                                    nc.tensor.matmul(
                                        pss[i], lhsT=w1s[c][:, msl],
                                        rhs=xpart[:, c, :], start=False,
                                        stop=(last and c == KF - 1))
                            nc.vector.tensor_scalar(
                                out=s1[:, m, :], in0=pss[i],
                                scalar1=d1s[:, m:m + 1],
                                scalar2=None, op0=ALU.is_ge)
                    if t == 0 and g == 0:
                        for i, m in enumerate(ms):
                            nc.vector.tensor_scalar(
                                out=s1[:, m, :], in0=pss[i],
                                scalar1=d1s[:, m:m + 1],
                                scalar2=None, op0=ALU.is_ge)

                # fc2 (fp8 {0,1} x +-2 exact, DoubleRow: 2 K-chunks/matmul)
                for m in range(C2):
                    msl = slice(m * 128, (m + 1) * 128)
                    ps = pmain.tile([128, NT], dt.float32, tag="ps",
                                    name=f"ps2_{t}_{m}")
                    for k in range(C1 // 2):
                        nc.tensor.matmul(ps, lhsT=w2s[k][:, :, msl],
                                         rhs=s1[:, 2 * k:2 * k + 2, :],
                                         start=(k == 0),
                                         stop=(k == C1 // 2 - 1),
                                         perf_mode=DR)
                    nc.vector.tensor_scalar(out=s2[:, m, :], in0=ps,
                                            scalar1=d2s[:, m:m + 1],
                                            scalar2=None, op0=ALU.is_ge)

                # fc3 (DoubleRow) + BN3 affine + hardtanh (bf16 out) on DVE
                for m in range(C3):
                    msl = slice(m * 128, (m + 1) * 128)
                    ps = pmain.tile([128, NT], dt.float32, tag="ps",
                                    name=f"ps3_{t}_{m}")
                    for k in range(C2 // 2):
                        nc.tensor.matmul(ps, lhsT=w3s[k][:, :, msl],
                                         rhs=s2[:, 2 * k:2 * k + 2, :],
                                         start=(k == 0),
                                         stop=(k == C2 // 2 - 1),
                                         perf_mode=DR)
                    bn3 = spool.tile([128, NT], dt.float32, tag="bn3",
                                     name=f"bn3_{t}_{m}")
                    nc.vector.tensor_scalar(out=bn3, in0=ps,
                                            scalar1=a3s[:, m:m + 1],
                                            scalar2=c3s[:, m:m + 1],
                                            op0=ALU.mult, op1=ALU.add)
                    nc.vector.tensor_scalar(out=h3[:, m, :], in0=bn3,
                                            scalar1=-1.0, scalar2=1.0,
                                            op0=ALU.max, op1=ALU.min)

                # fc4 (stationary = activations, moving = w4 hi|lo; b4 on
                # DVE) + log_softmax, as a short pipelined chain per 128-row
                # sub-tile; one output DMA per tile in [p][s][10] layout.
                osb3 = spool.tile([128, nsub, 10], dt.float32, tag="osb",
                                  name=f"osb_{t}", bufs=2)
                lg3 = spool.tile([128, nsub, 10], dt.float32, tag="lg3",
                                 name=f"lg3_{t}", bufs=2)
                for s in range(nsub):
                    ps4 = plog.tile([128, 20], dt.float32, tag="ps4",
                                    name=f"ps4_{t}_{s}")
                    ssl = slice(s * 128, (s + 1) * 128)
                    for c in range(C3):
                        nc.tensor.matmul(ps4, lhsT=h3[:, c, ssl],
                                         rhs=w4s[:, c, :],
                                         start=(c == 0), stop=(c == C3 - 1))
                    # DVE cannot read two PSUM operands; fold the b4 add
                    # into the lo-half staging copy
                    cp1 = spool.tile([128, 10], dt.float32, tag="cp1",
                                     name=f"cp1_{t}_{s}", bufs=2)
                    nc.vector.tensor_tensor(out=cp1, in0=ps4[:, 10:20],
                                            in1=b4s, op=ALU.add)
                    nc.vector.tensor_tensor(out=lg3[:, s, :],
                                            in0=ps4[:, 0:10],
                                            in1=cp1, op=ALU.add)
                # logits are bounded (|h3|<=1, small w4), so exp without
                # max-subtraction is safe.  One batched Exp over all 4
                # sub-tiles, per-sub sums on DVE, one batched Ln.
                ex3 = spool.tile([128, nsub, 10], dt.float32, tag="ex3",
                                 name=f"ex3_{t}", bufs=2)
                nc.scalar.activation(out=ex3, in_=lg3, func=AF.Exp)
                ssum3 = spool.tile([128, nsub, 1], dt.float32, tag="ssum3",
                                   name=f"ssum3_{t}", bufs=2)
                nc.vector.reduce_sum(out=ssum3, in_=ex3,
                                     axis=mybir.AxisListType.X)
                lns3 = spool.tile([128, nsub, 1], dt.float32, tag="lns3",
                                  name=f"lns3_{t}", bufs=2)
                nc.scalar.activation(out=lns3, in_=ssum3, func=AF.Ln)
                for s in range(nsub):
                    nc.vector.tensor_scalar(out=osb3[:, s, :],
                                            in0=lg3[:, s, :],
                                            scalar1=lns3[:, s, :],
                                            scalar2=None, op0=ALU.subtract)
                nc.sync.dma_start(out=out_d[t, :, :, :], in_=osb3)
    if do_compile:
        # bacc lowering: splits multi-waits into event semaphores (TRN2
        # allows only one sync wait per instruction), register alloc, etc.
        nc.compile()
    return nc


TRACE = False
_LAST_RESULT = [None]


def kernel(**inputs):
    from concourse.bass_utils import run_bass_kernel_spmd

    inp = {k: np.asarray(v) for k, v in inputs.items()}
    x = inp["x"].astype(np.float32)
    shared = _prep_shared(inp)
    nc = _build()
    in_maps = []
    for core in range(NCORES):
        m = _prep_x(x, core)
        m.update(shared)
        in_maps.append(m)
    res = run_bass_kernel_spmd(nc, in_maps, core_ids=list(range(NCORES)),
                               trace=TRACE)
    _LAST_RESULT[0] = res
    parts = []
    for r in res.results:
        o = np.asarray(r["out"], np.float32)       # [nbt, 128, nsub, 10]
        parts.append(o.transpose(0, 2, 1, 3).reshape(BC, 10))
    return np.concatenate(parts, axis=0)


# revision 18
# speedup vs baseline: 1.0768x; 1.0090x over previous
"""Trainium2 Bass kernel for nn_Net_3582002725506.

Binarized 4-layer MLP (eval mode):
  fc1(784->3072, sign weights) -> BN -> hardtanh
  fc2(3072->1536, sign both)   -> BN -> hardtanh
  fc3(1536->768, sign both)    -> BN -> hardtanh
  fc4(768->10, float)          -> log_softmax

Strategy: data-parallel batch shard across 8 cores (2048 rows each).
Activations kept transposed on-chip: [features(partitions), batch(free)].

Host-side prep (free, not on HW clock):
  - fc1: x split into 2 fp16 terms, hi = fp16(x) and lo = fp16(x - hi);
    both passes reuse the SAME +-1 fp8 sign-weight tiles (the PE handles
    fp16 subnormals exactly -- probed).  Combined representation error
    ~2^-23|x|, below fp32 PSUM accumulation noise, so numerically
    equivalent to the exact fp32 reference (1 borderline sign flip over
    the whole batch, final rel err ~1.4e-3 vs the 2e-2 budget).
    fc1 runs chunk-major over groups of 4 m-tiles so matmuls unlock as
    weight chunks stream in at startup; the 784 = 6*128 + 16 contraction
    remainder (hi rows 0..15, lo rows 16..31) is replicated across the
    4 partition quadrants and the 4 m-tiles' K=32 tail matmuls run
    CONCURRENTLY on disjoint PE row-groups via tile_position.
  - binarization via DVE is_ge -> u in {0,1} (instead of ScalarE Sign):
    next layer's weights are 2*sign(w) (+-2 exact in fp8) and the
    constant sum(w) row folds into the next threshold / BN3 bias.
    ScalarE then only ever runs Exp/Ln, so its two activation tables
    load once for the whole kernel (no per-tile table thrash).
  - BN1/BN2 + bias folded into per-feature threshold:
    u = (h >= -d), d = b - m + be/a, with sign(a) folded into the next
    layer's sign weights; BN3 kept affine (a3, c3) since fc4 consumes
    real values
  - fc4 bias b4 added on DVE via a broadcast tile; w4 split hi/lo bf16;
    log_softmax per 128-row sub-tile as a short pipelined chain, output
    written per-tile with a single DMA in [t][p][s][10] layout and
    rearranged on host.
"""

import numpy as np
import ml_dtypes

EPS = 1e-5
NCORES = 8
B = 16384
BC = B // NCORES            # 2048 rows per core
NT = 512                    # batch tile (matmul free dim / PSUM bank)
D0, D1, D2, D3 = 784, 3072, 1536, 768
KF = 6                      # full 128-row contraction chunks for fc1
KT = D0 - KF * 128          # 16-row tail
C1, C2, C3 = D1 // 128, D2 // 128, D3 // 128   # 24, 12, 6
MG = 4                      # fc1 m-tile group size (= open PSUM banks)

BF16 = ml_dtypes.bfloat16
FP8 = ml_dtypes.float8_e4m3
FP16 = np.float16


def _chunk3(a2d):
    """[K*128, M] -> [128, K, M] partition-major chunk layout (dtype kept)."""
    k = a2d.shape[0] // 128
    m = a2d.shape[1]
    return np.ascontiguousarray(a2d.reshape(k, 128, m).transpose(1, 0, 2))


def _split2(a):
    hi = a.astype(BF16)
    lo = (a - hi.astype(np.float32)).astype(BF16)
    return hi, lo


def _prep_shared(inp):
    """Host-side preprocessing of weights/BN params (shared by all cores)."""
    out = {}
    a1 = inp["g1"] / np.sqrt(inp["v1"] + EPS)
    a2 = inp["g2"] / np.sqrt(inp["v2"] + EPS)
    a3 = inp["g3"] / np.sqrt(inp["v3"] + EPS)

    # fc1 weights: sign + transpose; 6 full chunks shared by hi/lo passes.
    # 16-row tail: hi rows 0..15 + lo rows 16..31, replicated over the 4
    # partition quadrants for the tile_position-packed tail matmuls.
    s1w_t = np.sign(inp["w1"]).T.astype(np.float32)          # [784, 3072]
    out["w1t"] = _chunk3(s1w_t[:KF * 128].astype(FP8))       # [128, 6, 3072]
    tailblk = np.concatenate([s1w_t[KF * 128:], s1w_t[KF * 128:]], axis=0)
    out["w1tail"] = np.ascontiguousarray(
        np.tile(tailblk, (MG, 1)).astype(FP8))               # [128, 3072]

    # fc2/fc3 weights: 2*sign(w) (exact in fp8) with sign(a_prev) folded;
    # the {0,1} activation trick adds a constant row-sum per feature that
    # folds into the next threshold (d2) / BN3 bias (c3).
    s2w_t = (np.sign(inp["w2"]) * np.sign(a1)[None, :]).T    # [3072, 1536]
    out["w2t"] = _chunk3((2.0 * s2w_t).astype(FP8))          # [128, 24, 1536]
    row2 = s2w_t.sum(axis=0).astype(np.float32)              # [1536]
    s3w_t = (np.sign(inp["w3"]) * np.sign(a2)[None, :]).T    # [1536, 768]
    out["w3t"] = _chunk3((2.0 * s3w_t).astype(FP8))          # [128, 12, 768]
    row3 = s3w_t.sum(axis=0).astype(np.float32)              # [768]

    # fc4: [768, 10] hi/lo -> [128, 6, 20]
    w4hi, w4lo = _split2(inp["w4"].T.astype(np.float32))
    out["w4t"] = _chunk3(np.concatenate([w4hi, w4lo], axis=1))
    # bias as a 128-row broadcast tile for the DVE add
    out["b4bc"] = np.ascontiguousarray(
        np.broadcast_to(inp["b4"].astype(np.float32)[None, :], (128, 10)))

    # thresholds for the is_ge binarization: u = (psum >= thr)
    # fc1: h1 + d1 >= 0  ->  thr1 = -d1
    # fc2: psum2 = h2 + row2  ->  thr2 = row2 - d2
    d1 = (inp["b1"] - inp["m1"] + inp["be1"] / a1).astype(np.float32)
    d2 = (inp["b2"] - inp["m2"] + inp["be2"] / a2).astype(np.float32)
    out["d1"] = np.ascontiguousarray((-d1).reshape(C1, 128).T)  # [128, 24]
    out["d2"] = np.ascontiguousarray((row2 - d2).reshape(C2, 128).T)

    # BN3 affine on psum3 = h3 + row3: a3*(ps - row3) + c3
    c3 = (a3 * (inp["b3"] - inp["m3"] - row3) + inp["be3"]).astype(np.float32)
    out["a3"] = np.ascontiguousarray(a3.astype(np.float32).reshape(C3, 128).T)
    out["c3"] = np.ascontiguousarray(c3.reshape(C3, 128).T)  # [128, 6]
    return out


def _prep_x(x, core):
    """Per-core x shard -> transposed fp16 hi/lo split + packed tail."""
    xs = x[core * BC:(core + 1) * BC]                        # [2048, 784]
    xt = xs.T.astype(np.float32)                             # [784, 2048]
    hi = xt.astype(FP16)
    lo = (xt - hi.astype(np.float32)).astype(FP16)
    d = {}
    d["xh"] = _chunk3(hi[:KF * 128])                         # [128, 6, 2048]
    d["xl"] = _chunk3(lo[:KF * 128])
    tailblk = np.concatenate([hi[KF * 128:], lo[KF * 128:]], axis=0)
    d["xtail"] = np.ascontiguousarray(
        np.tile(tailblk, (MG, 1)))                           # [128, 2048]
    return d


def _build(bc=BC, do_compile=True):
    """Emit the Bass/Tile program (same program for all 8 cores)."""
    import concourse.mybir as mybir
    import concourse.tile as tile
    from concourse import bacc

    dt = mybir.dt
    AF = mybir.ActivationFunctionType
    ALU = mybir.AluOpType
    DR = mybir.MatmulPerfMode.DoubleRow

    nbt = bc // NT
    nsub = NT // 128

    nc = bacc.Bacc(trn_type="TRN2")
    xh_d = nc.declare_dram_parameter("xh", [128, KF, bc], dt.float16, False)
    xl_d = nc.declare_dram_parameter("xl", [128, KF, bc], dt.float16, False)
    xt_d = nc.declare_dram_parameter("xtail", [128, bc], dt.float16, False)
    w1_d = nc.declare_dram_parameter("w1t", [128, KF, D1], dt.float8e4, False)
    w1t_d = nc.declare_dram_parameter("w1tail", [128, D1], dt.float8e4, False)
    w2_d = nc.declare_dram_parameter("w2t", [128, C1, D2], dt.float8e4, False)
    w3_d = nc.declare_dram_parameter("w3t", [128, C2, D3], dt.float8e4, False)
    w4_d = nc.declare_dram_parameter("w4t", [128, C3, 20], dt.bfloat16, False)
    b4_d = nc.declare_dram_parameter("b4bc", [128, 10], dt.float32, False)
    d1_d = nc.declare_dram_parameter("d1", [128, C1], dt.float32, False)
    d2_d = nc.declare_dram_parameter("d2", [128, C2], dt.float32, False)
    a3_d = nc.declare_dram_parameter("a3", [128, C3], dt.float32, False)
    c3_d = nc.declare_dram_parameter("c3", [128, C3], dt.float32, False)
    out_d = nc.declare_dram_parameter("out", [nbt, 128, nsub, 10], dt.float32,
                                      True)

    with tile.TileContext(nc) as tc:
        with (
            tc.tile_pool(name="wpool", bufs=1) as wpool,
            tc.tile_pool(name="vpool", bufs=1) as vpool,
            tc.tile_pool(name="xpool", bufs=2) as xpool,
            tc.tile_pool(name="apool", bufs=1) as apool,
            tc.tile_pool(name="spool", bufs=3) as spool,
            tc.tile_pool(name="pmain", bufs=5, space="PSUM") as pmain,
            tc.tile_pool(name="plog", bufs=3, space="PSUM") as plog,
        ):
            # PE warm-up: dummy matmuls (rotating the main PSUM banks so
            # they pipeline) keep the PE busy while the first DMAs land,
            # opening the HAM clock-gate (1.2 -> 2.4 GHz) before real work.
            warm_src = vpool.tile([128, NT], dt.bfloat16)
            nc.vector.memset(warm_src, 0.0)
            for i in range(16):
                wps = pmain.tile([128, NT], dt.float32, tag="ps",
                                 name=f"wps_{i}")
                nc.tensor.matmul(wps, lhsT=warm_src[:, 0:128], rhs=warm_src,
                                 start=True, stop=True)

            def alloc_x(t):
                tiles = []
                for nm in ("xh", "xl"):
                    tiles.append(xpool.tile([128, KF, NT], dt.float16,
                                            tag=nm, name=f"{nm}_{t}"))
                tiles.append(xpool.tile([128, NT], dt.float16, tag="xt",
                                        name=f"xt_{t}"))
                return tiles

            def dma_x(t, tiles):
                sl = slice(t * NT, (t + 1) * NT)
                for p, src in zip(tiles, (xh_d, xl_d)):
                    nc.sync.dma_start(out=p, in_=src[:, :, sl])
                nc.sync.dma_start(out=tiles[2], in_=xt_d[:, sl])

            def load_x(t):
                tiles = alloc_x(t)
                dma_x(t, tiles)
                return tiles

            # startup-critical-path DMA order: fc1 runs chunk-major, so it
            # needs xh + w1 chunks in order, then xl, tails, thresholds.
            xt = [None] * nbt
            x0 = alloc_x(0)
            xt[0] = x0
            sl0 = slice(0, NT)
            nc.sync.dma_start(out=x0[0], in_=xh_d[:, :, sl0])
            nc.sync.dma_start(out=x0[2], in_=xt_d[:, sl0])
            w1tl = wpool.tile([128, D1], dt.float8e4)
            nc.sync.dma_start(out=w1tl, in_=w1t_d[:, :])
            w1s = []
            for c in range(KF):
                w = wpool.tile([128, D1], dt.float8e4, tag=f"w1_{c}",
                               name=f"w1_{c}")
                w1s.append(w)
            nc.sync.dma_start(out=w1s[0], in_=w1_d[:, 0, :])
            nc.sync.dma_start(out=w1s[1], in_=w1_d[:, 1, :])
            nc.sync.dma_start(out=x0[1], in_=xl_d[:, :, sl0])
            nc.sync.dma_start(out=w1s[2], in_=w1_d[:, 2, :])
            nc.sync.dma_start(out=w1s[3], in_=w1_d[:, 3, :])
            nc.sync.dma_start(out=w1s[4], in_=w1_d[:, 4, :])
            nc.sync.dma_start(out=w1s[5], in_=w1_d[:, 5, :])
            # startup group-0 matmul order: hi c0, c1, then lo c0, c1 (after
            # xl lands), then hi/lo alternating per chunk -- keeps the PE
            # fed at the pace weight chunks stream in.
            g0_order = [(0, 0), (0, 1), (1, 0), (1, 1)] + [
                (hl, c) for c in range(2, KF) for hl in (0, 1)]
            d1s = vpool.tile([128, C1], dt.float32)
            nc.sync.dma_start(out=d1s, in_=d1_d[:, :])
            d2s = vpool.tile([128, C2], dt.float32)
            nc.sync.dma_start(out=d2s, in_=d2_d[:, :])
            a3s = vpool.tile([128, C3], dt.float32)
            nc.sync.dma_start(out=a3s, in_=a3_d[:, :])
            c3s = vpool.tile([128, C3], dt.float32)
            nc.sync.dma_start(out=c3s, in_=c3_d[:, :])
            b4s = vpool.tile([128, 10], dt.float32)
            nc.sync.dma_start(out=b4s, in_=b4_d[:, :])
            w2s = []
            for k in range(C1 // 2):
                w = wpool.tile([128, 2, D2], dt.float8e4, tag=f"w2_{k}",
                               name=f"w2_{k}")
                nc.sync.dma_start(out=w, in_=w2_d[:, 2 * k:2 * k + 2, :])
                w2s.append(w)
            w3s = []
            for k in range(C2 // 2):
                w = wpool.tile([128, 2, D3], dt.float8e4, tag=f"w3_{k}",
                               name=f"w3_{k}")
                nc.sync.dma_start(out=w, in_=w3_d[:, 2 * k:2 * k + 2, :])
                w3s.append(w)
            w4s = wpool.tile([128, C3, 20], dt.bfloat16)
            nc.sync.dma_start(out=w4s, in_=w4_d[:, :, :])

            for t in range(nbt):
                if t + 1 < nbt:
                    xt[t + 1] = load_x(t + 1)
                xh, xl, xtl = xt[t]
                s1 = apool.tile([128, C1, NT], dt.float8e4, tag="s1",
                                name=f"s1_{t}")
                s2 = apool.tile([128, C2, NT], dt.float8e4, tag="s2",
                                name=f"s2_{t}")
                h3 = apool.tile([128, C3, NT], dt.bfloat16, tag="h3",
                                name=f"h3_{t}")

                # fc1 over groups of MG m-tiles: the 4 K=32 tail matmuls go
                # FIRST (start=True), packed concurrently on disjoint PE
                # row-groups via tile_position; then the hi/lo main matmuls.
                # Steady state runs m-major so each m-tile's PSUM closes
                # early and its DVE binarization overlaps the next m-tile's
                # matmuls (no bank-recycle stall); the startup group runs
                # chunk-major so matmuls unlock as weight chunks stream in.
                for g in range(C1 // MG):
                    ms = range(g * MG, (g + 1) * MG)
                    pss = [pmain.tile([128, NT], dt.float32, tag="ps",
                                      name=f"ps1_{t}_{m}") for m in ms]
                    for i, m in enumerate(ms):
                        msl = slice(m * 128, (m + 1) * 128)
                        nc.tensor.matmul(
                            pss[i], lhsT=w1tl[32 * i:32 * i + 32, msl],
                            rhs=xtl[32 * i:32 * i + 32, :],
                            start=True, stop=False,
                            tile_position=(32 * i, 0))
                    if t == 0 and g == 0:
                        for hl, c in g0_order:
                            xpart = xh if hl == 0 else xl
                            last = (hl, c) == g0_order[-1]
                            for i, m in enumerate(ms):
                                msl = slice(m * 128, (m + 1) * 128)
                                nc.tensor.matmul(
                                    pss[i], lhsT=w1s[c][:, msl],
                                    rhs=xpart[:, c, :], start=False,
                                    stop=last)
                    else:
                        for i, m in enumerate(ms):
                            msl = slice(m * 128, (m + 1) * 128)
                            for xpart, last in ((xh, False), (xl, True)):
                                for c in range(KF):
                                    nc.tensor.matmul(
                                        pss[i], lhsT=w1s[c][:, msl],
                                        rhs=xpart[:, c, :], start=False,
                                        stop=(last and c == KF - 1))
                            nc.vector.tensor_scalar(
                                out=s1[:, m, :], in0=pss[i],
                                scalar1=d1s[:, m:m + 1],
                                scalar2=None, op0=ALU.is_ge)
                    if t == 0 and g == 0:
                        for i, m in enumerate(ms):
                            nc.vector.tensor_scalar(
                                out=s1[:, m, :], in0=pss[i],
                                scalar1=d1s[:, m:m + 1],
                                scalar2=None, op0=ALU.is_ge)

                # fc2 (fp8 {0,1} x +-2 exact, DoubleRow: 2 K-chunks/matmul)
                for m in range(C2):
                    msl = slice(m * 128, (m + 1) * 128)
                    ps = pmain.tile([128, NT], dt.float32, tag="ps",
                                    name=f"ps2_{t}_{m}")
                    for k in range(C1 // 2):
                        nc.tensor.matmul(ps, lhsT=w2s[k][:, :, msl],
                                         rhs=s1[:, 2 * k:2 * k + 2, :],
                                         start=(k == 0),
                                         stop=(k == C1 // 2 - 1),
                                         perf_mode=DR)
                    nc.vector.tensor_scalar(out=s2[:, m, :], in0=ps,
                                            scalar1=d2s[:, m:m + 1],
                                            scalar2=None, op0=ALU.is_ge)

                # fc3 (DoubleRow) + BN3 affine + hardtanh (bf16 out) on DVE
                for m in range(C3):
                    msl = slice(m * 128, (m + 1) * 128)
                    ps = pmain.tile([128, NT], dt.float32, tag="ps",
                                    name=f"ps3_{t}_{m}")
                    for k in range(C2 // 2):
                        nc.tensor.matmul(ps, lhsT=w3s[k][:, :, msl],
                                         rhs=s2[:, 2 * k:2 * k + 2, :],
                                         start=(k == 0),
                                         stop=(k == C2 // 2 - 1),
                                         perf_mode=DR)
                    bn3 = spool.tile([128, NT], dt.float32, tag="bn3",
                                     name=f"bn3_{t}_{m}")
                    nc.vector.tensor_scalar(out=bn3, in0=ps,
                                            scalar1=a3s[:, m:m + 1],
                                            scalar2=c3s[:, m:m + 1],
                                            op0=ALU.mult, op1=ALU.add)
                    nc.vector.tensor_scalar(out=h3[:, m, :], in0=bn3,
                                            scalar1=-1.0, scalar2=1.0,
                                            op0=ALU.max, op1=ALU.min)

                # fc4 (stationary = activations, moving = w4 hi|lo; b4 on
                # DVE) + log_softmax, as a short pipelined chain per 128-row
                # sub-tile; one output DMA per tile in [p][s][10] layout.
                osb3 = spool.tile([128, nsub, 10], dt.float32, tag="osb",
                                  name=f"osb_{t}", bufs=2)
                lg3 = spool.tile([128, nsub, 10], dt.float32, tag="lg3",
                                 name=f"lg3_{t}", bufs=2)
                for s in range(nsub):
                    ps4 = plog.tile([128, 20], dt.float32, tag="ps4",
                                    name=f"ps4_{t}_{s}")
                    ssl = slice(s * 128, (s + 1) * 128)
                    for c in range(C3):
                        nc.tensor.matmul(ps4, lhsT=h3[:, c, ssl],
                                         rhs=w4s[:, c, :],
                                         start=(c == 0), stop=(c == C3 - 1))
                    # DVE cannot read two PSUM operands; fold the b4 add
                    # into the lo-half staging copy
                    cp1 = spool.tile([128, 10], dt.float32, tag="cp1",
                                     name=f"cp1_{t}_{s}", bufs=2)
                    nc.vector.tensor_tensor(out=cp1, in0=ps4[:, 10:20],
                                            in1=b4s, op=ALU.add)
                    nc.vector.tensor_tensor(out=lg3[:, s, :],
                                            in0=ps4[:, 0:10],
                                            in1=cp1, op=ALU.add)
                # logits are bounded (|h3|<=1, small w4), so exp without
                # max-subtraction is safe.  One batched Exp over all 4
                # sub-tiles, per-sub sums on DVE.  ln(sum) is computed on
                # DVE via exponent/mantissa bit extraction + a degree-4
                # minimax polynomial (max err 1.4e-4) so ScalarE only ever
                # runs Exp -- its activation table loads once per kernel
                # instead of twice per tile (the reloads sat on the exposed
                # last-tile critical path).
                ex3 = spool.tile([128, nsub, 10], dt.float32, tag="ex3",
                                 name=f"ex3_{t}", bufs=2)
                nc.scalar.activation(out=ex3, in_=lg3, func=AF.Exp)
                ssum3 = spool.tile([128, nsub, 1], dt.float32, tag="ssum3",
                                   name=f"ssum3_{t}", bufs=2)
                nc.vector.reduce_sum(out=ssum3, in_=ex3,
                                     axis=mybir.AxisListType.X)
                su = ssum3[:, :, :].bitcast(dt.uint32)
                eint = spool.tile([128, nsub, 1], dt.uint32, tag="eint",
                                  name=f"eint_{t}", bufs=2)
                nc.vector.tensor_scalar(out=eint, in0=su, scalar1=23,
                                        scalar2=None,
                                        op0=ALU.logical_shift_right)
                LN2 = 0.6931471805599453
                LC0, LC1, LC2, LC3, LC4 = (-1.7306317, 2.79225523,
                                           -1.44248101, 0.43586185,
                                           -0.05486285)
                eln = spool.tile([128, nsub, 1], dt.float32, tag="eln",
                                 name=f"eln_{t}", bufs=2)
                nc.vector.tensor_scalar(out=eln, in0=eint, scalar1=LN2,
                                        scalar2=LC0 - 127.0 * LN2,
                                        op0=ALU.mult, op1=ALU.add)
                mu = spool.tile([128, nsub, 1], dt.uint32, tag="mu",
                                name=f"mu_{t}", bufs=2)
                nc.vector.tensor_scalar(out=mu, in0=su, scalar1=0x007FFFFF,
                                        scalar2=0x3F800000,
                                        op0=ALU.bitwise_and,
                                        op1=ALU.bitwise_or)
                # Horner with the (p + c) * m primitive: start p0 = LC4*m,
                # then p_{k} = (p_{k-1} + LC_k) * m, so
                # p3 = LC1*m + LC2*m^2 + LC3*m^3 + LC4*m^4 (LC0 sits in eln).
                mf = mu[:, :, :].bitcast(dt.float32)
                p0 = spool.tile([128, nsub, 1], dt.float32, tag="p0",
                                name=f"p0_{t}", bufs=2)
                nc.vector.tensor_scalar(out=p0, in0=mf, scalar1=LC4,
                                        scalar2=None, op0=ALU.mult)
                p1 = spool.tile([128, nsub, 1], dt.float32, tag="p1",
                                name=f"p1_{t}", bufs=2)
                nc.vector.scalar_tensor_tensor(out=p1, in0=p0, scalar=LC3,
                                               in1=mf, op0=ALU.add,
                                               op1=ALU.mult)
                p2 = spool.tile([128, nsub, 1], dt.float32, tag="p2",
                                name=f"p2_{t}", bufs=2)
                nc.vector.scalar_tensor_tensor(out=p2, in0=p1, scalar=LC2,
                                               in1=mf, op0=ALU.add,
                                               op1=ALU.mult)
                p3 = spool.tile([128, nsub, 1], dt.float32, tag="p3",
                                name=f"p3_{t}", bufs=2)
                nc.vector.scalar_tensor_tensor(out=p3, in0=p2, scalar=LC1,
                                               in1=mf, op0=ALU.add,
                                               op1=ALU.mult)
                lns3 = spool.tile([128, nsub, 1], dt.float32, tag="lns3",
                                  name=f"lns3_{t}", bufs=2)
                nc.vector.tensor_tensor(out=lns3, in0=p3, in1=eln,
                                        op=ALU.add)
                for s in range(nsub):
                    nc.vector.tensor_scalar(out=osb3[:, s, :],
                                            in0=lg3[:, s, :],
                                            scalar1=lns3[:, s, :],
                                            scalar2=None, op0=ALU.subtract)
                nc.sync.dma_start(out=out_d[t, :, :, :], in_=osb3)
    if do_compile:
        # bacc lowering: splits multi-waits into event semaphores (TRN2
        # allows only one sync wait per instruction), register alloc, etc.
        nc.compile()
    return nc


TRACE = False
_LAST_RESULT = [None]


def kernel(**inputs):
    from concourse.bass_utils import run_bass_kernel_spmd

    inp = {k: np.asarray(v) for k, v in inputs.items()}
    x = inp["x"].astype(np.float32)
    shared = _prep_shared(inp)
    nc = _build()
    in_maps = []
    for core in range(NCORES):
        m = _prep_x(x, core)
        m.update(shared)
        in_maps.append(m)
    res = run_bass_kernel_spmd(nc, in_maps, core_ids=list(range(NCORES)),
                               trace=TRACE)
    _LAST_RESULT[0] = res
    parts = []
    for r in res.results:
        o = np.asarray(r["out"], np.float32)       # [nbt, 128, nsub, 10]
        parts.append(o.transpose(0, 2, 1, 3).reshape(BC, 10))
    return np.concatenate(parts, axis=0)
